# revision 24
# baseline (speedup 1.0000x reference)
"""Multi-head attention (B=4, L=2048, E=1024, H=16, DK=64) on 8 TRN2 cores.

Sharding: core c -> (batch b = c//2, head-group g = c%2 of 8 heads).

Single fused instruction stream per core: one software pipeline over 256
attention steps (4 query-quarters x 4 head-pairs x 16 key-tiles). Every
matmul in the kernel is a PE tile-shape (64,128) op, and consecutive
matmuls are paired on complementary partition halves (positions (0,0) /
(64,0)) with identical moving columns so the PE fuses each pair into a
single 512-cycle pass. Per step: fused ST pass (2 heads) -> exp[128,1024]
(ACT) -> 2 fused AV passes (key-halves accumulate into one PSUM bank via
two fixed-position groups). QKV/FC chains drip into the stream as
background PE work; per-quarter partial FC outputs are pairwise
ReduceScattered (bf16) while later quarters compute. Host casts to f32.

Self-contained: hardcodes all shapes; requires only the concourse stack.
"""

import numpy as np
import ml_dtypes

try:
    import axon_prof

    axon_prof.install()
except Exception:
    pass

import concourse.mybir as mybir
import concourse.tile as tile
from concourse import bacc
from concourse import bass_utils

B, L, E = 4, 2048, 1024
H, DK = 16, 64
H8 = 8                      # heads per core
F = H8 * 3 * DK             # qkv features per core = 1536
FO = H8 * DK                # attn-out features per core = 512
NCORES = 8
Q4 = L // 4                 # 512 queries per quarter
Q8 = Q4 // 2                # 256 tokens scattered to each pair member

# ft-tile order: Q tiles 0..3 (head-pairs), K tiles 4..7, V tiles 8..11.
# Tiles {0,4,8} (head-pair 0) ship in w_pre so block (0,0) starts early.
PRE_FTS = (0, 4, 8)
REST_FTS = (1, 2, 3, 5, 6, 7, 9, 10, 11)
PRE_IDX = {ft: i for i, ft in enumerate(PRE_FTS)}
REST_IDX = {ft: i for i, ft in enumerate(REST_FTS)}

f32 = mybir.dt.float32
bf16 = mybir.dt.bfloat16
Exp = mybir.ActivationFunctionType.Exp
MUL = mybir.AluOpType.mult
ADD = mybir.AluOpType.add

_CACHE = {}


def build_nc():
    nc = bacc.Bacc("TRN2", target_bir_lowering=False, debug=False, num_devices=NCORES)

    x = nc.dram_tensor("x", [E, L], bf16, kind="ExternalInput")
    w_pre = nc.dram_tensor("w_pre", [E, 384], bf16, kind="ExternalInput")
    w_rest = nc.dram_tensor("w_rest", [E, 1152], bf16, kind="ExternalInput")
    b_qkv = nc.dram_tensor("b_qkv", [128, 12], f32, kind="ExternalInput")
    w_fc = nc.dram_tensor("w_fc", [FO, E], bf16, kind="ExternalInput")
    b_fc = nc.dram_tensor("b_fc", [1, E], f32, kind="ExternalInput")
    # 4 quarters x 256 scattered tokens each; host casts bf16 -> f32
    out = nc.dram_tensor("out", [L // 2, E], bf16, kind="ExternalOutput")

    PAIRS = [[0, 1], [2, 3], [4, 5], [6, 7]]

    with tile.TileContext(nc) as tc:
        with (
            tc.tile_pool(name="persist", bufs=1) as pp,
            tc.tile_pool(name="work", bufs=2) as wp,
            tc.tile_pool(name="ys", bufs=3) as yp_pool,
            tc.tile_pool(name="stp", bufs=2, space="PSUM") as stp,
            tc.tile_pool(name="avp", bufs=1, space="PSUM") as avp,
            tc.tile_pool(name="qp", bufs=1, space="PSUM") as qp,
            tc.tile_pool(name="dram", bufs=1, space="DRAM") as dram,
        ):
            # ---- persistent SBUF ----
            xT = pp.tile([128, 8, L], bf16, tag="xT")          # X^T  4 MiB
            wqp = pp.tile([128, 8, 384], bf16, tag="wqp")
            wqr = pp.tile([128, 8, 1152], bf16, tag="wqr")
            bq = pp.tile([128, 12], f32, tag="bq")
            wfc = pp.tile([128, 4, E], bf16, tag="wfc")        # 1 MiB
            bias = pp.tile([128, E], f32, tag="bias")          # 0.5 MiB
            qt = pp.tile([128, 4, L], bf16, tag="qt")          # Q^T 2 MiB
            kt = pp.tile([128, 4, L], bf16, tag="kt")          # K^T 2 MiB
            vt = pp.tile([128, 4, L], bf16, tag="vt")          # V^T 2 MiB
            # V natural layout, 80-elem stride; col 64 holds the ones column
            # so AV matmuls with lhsT [V|1] (M=65) produce rowsums for free
            v = pp.tile([128, H8, 16, 80], bf16, tag="v")      # 2.5 MiB
            onT = pp.tile([128, 4, L], bf16, tag="onT")        # attn out^T 2 MiB

            rs_in = [
                dram.tile([Q4, E], bf16, name=f"rs_in{i}", tag=f"rs_in{i}")
                for i in range(4)
            ]
            rs_out = [
                dram.tile([Q8, E], bf16, name=f"rs_out{i}", tag=f"rs_out{i}")
                for i in range(4)
            ]

            # ---- input DMAs, all on the sync (SP) queue: the ACT queue must
            # stay empty or exp issue stalls behind DMA triggers ----
            nc.sync.dma_start(wqp[:], w_pre.rearrange("(a p) f -> p a f", p=128))
            for e in range(8):
                nc.sync.dma_start(xT[:, e, :], x[e * 128 : (e + 1) * 128, :])
            nc.sync.dma_start(wqr[:], w_rest.rearrange("(a p) f -> p a f", p=128))
            nc.sync.dma_start(bq[:], b_qkv[:])
            nc.sync.dma_start(wfc[:], w_fc.rearrange("(c p) e -> p c e", p=128))
            bfc_row = pp.tile([1, E], f32, tag="bfc_row")
            nc.sync.dma_start(bfc_row[:], b_fc[:])
            nc.gpsimd.partition_broadcast(bias[:], bfc_row[:])
            nc.vector.memset(v[:, :, :, 64:65], 1.0)

            def wq_sl(ft, po2):
                if ft in PRE_IDX:
                    i = PRE_IDX[ft]
                    return lambda kc: wqp[po2 : po2 + 64, kc, i * 128 : (i + 1) * 128]
                i = REST_IDX[ft]
                return lambda kc: wqr[po2 : po2 + 64, kc, i * 128 : (i + 1) * 128]

            # ---- emitters: every matmul is a (64,128)-shape op; pairs at
            # positions (0,0)/(64,0) with identical moving columns fuse ----
            def qkv_chain(ft, tb):
                psA = qp.tile([128, 512], f32, tag="psA", name="psA")
                psB = qp.tile([128, 512], f32, tag="psB", name="psB")
                wa, wb = wq_sl(ft, 0), wq_sl(ft, 64)
                tsl = slice(tb * 512, (tb + 1) * 512)
                for kc in range(8):
                    nc.tensor.matmul(
                        psA[:], wa(kc), xT[0:64, kc, tsl],
                        start=(kc == 0), stop=(kc == 7),
                    )
                    nc.tensor.matmul(
                        psB[:], wb(kc), xT[64:128, kc, tsl],
                        start=(kc == 0), stop=(kc == 7),
                    )
                if ft < 4:
                    dst = qt[:, ft, tsl]
                elif ft < 8:
                    dst = kt[:, ft - 4, tsl]
                else:
                    dst = vt[:, ft - 8, tsl]
                nc.vector.tensor_scalar_add(dst, psA[:], bq[:, ft : ft + 1])
                nc.vector.tensor_tensor(dst, dst, psB[:], op=ADD)

            def v_transpose(p, tb):
                # V^T -> V (token-major) via xbar transpose, per head, per
                # 512-token slice (4 key-chunks)
                for h in (2 * p, 2 * p + 1):
                    nc.sync.dma_start_transpose(
                        v[:, h, tb * 4 : (tb + 1) * 4, 0:DK],
                        vt[(h % 2) * 64 : (h % 2) * 64 + 64, p, tb * 512 : (tb + 1) * 512],
                    )

            def fc_chain(qq, tc_i, e2):
                # tokens (qq*4 + tc_i)*128 .. +128, output cols e2*512 .. +512
                t0 = (qq * 4 + tc_i) * 128
                esl = slice(e2 * 512, (e2 + 1) * 512)
                yA = qp.tile([128, 512], f32, tag="psA", name="yA")
                yB = qp.tile([128, 512], f32, tag="psB", name="yB")
                for c in range(4):
                    nc.tensor.matmul(
                        yA[:], onT[0:64, c, t0 : t0 + 128], wfc[0:64, c, esl],
                        start=(c == 0), stop=(c == 3),
                    )
                    nc.tensor.matmul(
                        yB[:], onT[64:128, c, t0 : t0 + 128], wfc[64:128, c, esl],
                        start=(c == 0), stop=(c == 3),
                    )
                ys = yp_pool.tile([128, 512], bf16, tag="ys", name="ys")
                nc.vector.tensor_tensor(ys[:], yA[:], bias[:, esl], op=ADD)
                nc.vector.tensor_tensor(ys[:], ys[:], yB[:], op=ADD)
                nc.sync.dma_start(
                    rs_in[qq][tc_i * 128 : (tc_i + 1) * 128, esl], ys[:]
                )

            def rs_quarter(qq):
                nc.gpsimd.collective_compute(
                    "ReduceScatter",
                    ADD,
                    replica_groups=PAIRS,
                    ins=[rs_in[qq].opt()],
                    outs=[rs_out[qq].opt()],
                )
                # deferred: the SWDGE copy for the PREVIOUS quarter (its RS is
                # long done, so this trigger never blocks the Pool stream)
                if qq > 0:
                    nc.gpsimd.dma_start(
                        out[(qq - 1) * Q8 : qq * Q8, :], rs_out[qq - 1][:]
                    )

            # ---- background schedule (due_step, fn); run at END of step s ----
            bg = []
            for tb in (1, 2, 3):
                bg.append((4 * tb - 2, lambda tb=tb: qkv_chain(4, tb)))
                bg.append((4 * tb - 1, lambda tb=tb: (qkv_chain(8, tb), v_transpose(0, tb))))
            for p in (1, 2, 3):
                bg.append((16 * p - 4, lambda p=p: qkv_chain(p, 0)))
                for tb in range(4):
                    bg.append((16 * p + 4 * tb - 2, lambda p=p, tb=tb: qkv_chain(4 + p, tb)))
                    bg.append(
                        (16 * p + 4 * tb - 1,
                         lambda p=p, tb=tb: (qkv_chain(8 + p, tb), v_transpose(p, tb)))
                    )
            for tb in (1, 2, 3):
                for p in range(4):
                    bg.append((64 * tb + 16 * p - 4, lambda p=p, tb=tb: qkv_chain(p, tb)))
            for qq in range(3):
                items = [(tc_i, e2) for tc_i in range(4) for e2 in range(2)]
                for i, (tc_i, e2) in enumerate(items):
                    due = 64 * qq + 66 + 4 * i
                    is_last = i == len(items) - 1
                    def fitem(qq=qq, tc_i=tc_i, e2=e2, is_last=is_last):
                        fc_chain(qq, tc_i, e2)
                        if is_last:
                            rs_quarter(qq)
                    bg.append((due, fitem))
            bg.sort(key=lambda t: t[0])
            bg_i = [0]

            def run_due_bg(s):
                while bg_i[0] < len(bg) and bg[bg_i[0]][0] <= s:
                    bg[bg_i[0]][1]()
                    bg_i[0] += 1

            # ---- prelude: head-pair 0, first 512 tokens ----
            qkv_chain(0, 0)
            qkv_chain(4, 0)
            qkv_chain(8, 0)
            v_transpose(0, 0)

            # ---- attention pipeline: 256 steps (1 key-tile x 2 heads) ----
            pts = {}
            av_tiles = {}

            def emit_av(s2):
                b_i, kk = divmod(s2, 16)
                qq, j = divmod(b_i, 4)
                if kk == 0:
                    av_tiles[b_i] = (
                        avp.tile([128, 512], f32, tag="avh0", name="avh0"),
                        avp.tile([128, 512], f32, tag="avh1", name="avh1"),
                    )
                pt = pts.pop(s2)
                for u, av in zip((0, 1), av_tiles[b_i]):
                    h = 2 * j + u
                    usl = slice(u * 512, (u + 1) * 512)
                    nc.tensor.matmul(
                        av[0:65, :],
                        v[:, h, kk, 0:65],
                        pt[:, usl],
                        start=(kk == 0),
                        stop=(kk == 15),
                    )
                if kk == 15:
                    evict_block(b_i, *av_tiles.pop(b_i))

            def evict_block(b_i, a0, a1):
                qq, j = divmod(b_i, 4)
                qsl = slice(qq * Q4, (qq + 1) * Q4)
                # one PSUM-read per av bank frees them for the next block
                comb = wp.tile([128, 1024], f32, tag="comb", name="comb")
                nc.vector.tensor_copy(comb[0:65, 0:512], a0[0:65, :])
                nc.vector.tensor_copy(comb[0:65, 512:1024], a1[0:65, :])
                # sums sit on partition 64; shift to partition 0, reciprocal,
                # broadcast; normalize on gpsimd (SBUF-only ops) to spare DVE
                srs = wp.tile([128, 2048], f32, tag="srs", name="srs")
                nc.sync.dma_start(srs[0:1, 0:1024], comb[64:65, 0:1024])
                nc.vector.reciprocal_approx_fast(
                    srs[0:1, 1024:2048], srs[0:1, 0:1024]
                )
                R = wp.tile([128, 1024], f32, tag="R", name="R")
                nc.gpsimd.partition_broadcast(R[:], srs[0:1, 1024:2048])
                nc.gpsimd.tensor_tensor(
                    onT[0:64, j, qsl], comb[0:64, 0:512], R[0:64, 0:512], op=MUL
                )
                tmp = wp.tile([64, 512], bf16, tag="tmp", name="tmp")
                nc.gpsimd.tensor_tensor(
                    tmp[:], comb[0:64, 512:1024], R[0:64, 512:1024], op=MUL
                )
                nc.sync.dma_start(onT[64:128, j, qsl], tmp[:])

            for s in range(256):
                b_i, kk = divmod(s, 16)
                qq, j = divmod(b_i, 4)
                qsl = slice(qq * Q4, (qq + 1) * Q4)
                ksl = slice(kk * 128, (kk + 1) * 128)
                st = stp.tile([128, 1024], f32, tag="st", name="st")
                nc.tensor.matmul(
                    st[:, 0:512], kt[0:64, j, ksl], qt[0:64, j, qsl],
                    start=True, stop=True,
                )
                nc.tensor.matmul(
                    st[:, 512:1024], kt[64:128, j, ksl], qt[64:128, j, qsl],
                    start=True, stop=True,
                )
                pt = wp.tile([128, 1024], bf16, tag="pt", bufs=3, name="pt")
                nc.scalar.activation(pt[:], st[:], Exp, scale=0.125)
                pts[s] = pt
                if s > 0:
                    emit_av(s - 1)
                run_due_bg(s)
            emit_av(255)

            # ---- tail: FC + RS for the last quarter ----
            run_due_bg(10**9)
            for tc_i in range(4):
                for e2 in range(2):
                    fc_chain(3, tc_i, e2)
            rs_quarter(3)
            nc.gpsimd.dma_start(out[2 * Q8 : 3 * Q8, :], rs_out[2][:])
            nc.gpsimd.dma_start(out[3 * Q8 : 4 * Q8, :], rs_out[3][:])

    nc.finalize()
    return nc


def _prep_inputs(X, W_qkv, b_qkv, W_fc, b_fc):
    """Host-side shard + permute + cast. Returns in_maps for 8 cores."""
    X = np.asarray(X, dtype=np.float32)
    W_qkv = np.asarray(W_qkv, dtype=np.float32)
    b_qkv = np.asarray(b_qkv, dtype=np.float32)
    W_fc = np.asarray(W_fc, dtype=np.float32)
    b_fc = np.asarray(b_fc, dtype=np.float32)

    pre_cols = np.concatenate([np.arange(ft * 128, (ft + 1) * 128) for ft in PRE_FTS])
    rest_cols = np.concatenate([np.arange(ft * 128, (ft + 1) * 128) for ft in REST_FTS])

    in_maps = []
    bfc_half = (0.5 * b_fc).astype(np.float32).reshape(1, E)
    for c in range(NCORES):
        b, g = divmod(c, 2)
        heads = np.arange(g * H8, (g + 1) * H8)
        # column order: all Q feats (head-major), then K, then V
        cols = np.concatenate(
            [
                np.concatenate([h * 3 * DK + off + np.arange(DK) for h in heads])
                for off in (0, DK, 2 * DK)
            ]
        )
        wq_sh = W_qkv[:, cols].astype(ml_dtypes.bfloat16)
        bq_sh = b_qkv[cols].astype(np.float32).reshape(12, 128).T.copy()
        wfc_sh = W_fc[g * FO : (g + 1) * FO, :].astype(ml_dtypes.bfloat16)
        in_maps.append(
            {
                "x": np.ascontiguousarray(X[b].T).astype(ml_dtypes.bfloat16),
                "w_pre": np.ascontiguousarray(wq_sh[:, pre_cols]),
                "w_rest": np.ascontiguousarray(wq_sh[:, rest_cols]),
                "b_qkv": np.ascontiguousarray(bq_sh),
                "w_fc": wfc_sh,
                "b_fc": bfc_half,
            }
        )
    return in_maps


def run_kernel(inputs, trace=False):
    if "nc" not in _CACHE:
        _CACHE["nc"] = build_nc()
    nc = _CACHE["nc"]
    in_maps = _prep_inputs(**inputs)
    res = bass_utils.run_bass_kernel_spmd(
        nc, in_maps, core_ids=list(range(NCORES)), trace=trace
    )
    Y = np.empty((B, L, E), dtype=np.float32)
    for c in range(NCORES):
        b, g = divmod(c, 2)
        o = np.asarray(res.results[c]["out"]).astype(np.float32)
        for qq in range(4):
            Y[b, qq * Q4 + g * Q8 : qq * Q4 + (g + 1) * Q8, :] = o[
                qq * Q8 : (qq + 1) * Q8
            ]
    return Y, res


def kernel(X, W_qkv, b_qkv, W_fc, b_fc):
    Y, _ = run_kernel(
        dict(X=X, W_qkv=W_qkv, b_qkv=b_qkv, W_fc=W_fc, b_fc=b_fc), trace=False
    )
    return Y


# revision 33
# speedup vs baseline: 1.1546x; 1.1546x over previous
"""Multi-head attention (B=4, L=2048, E=1024, H=16, DK=64) on 8 TRN2 cores.

Sharding: core c -> (batch b = c//2, head-group g = c%2 of 8 heads).

Single fused instruction stream per core: one software pipeline over 256
attention steps (4 query-quarters x 4 head-pairs x 16 key-tiles). Every
matmul in the kernel is a PE tile-shape (64,128) op, and consecutive
matmuls are paired on complementary partition halves (positions (0,0) /
(64,0)) with identical moving columns so the PE fuses each pair into a
single 512-cycle pass. Per step: fused ST pass (2 heads) -> exp[128,1024]
(ACT) -> 2 fused AV passes (key-halves accumulate into one PSUM bank via
two fixed-position groups). QKV/FC chains drip into the stream as
background PE work; per-quarter partial FC outputs are pairwise
ReduceScattered (bf16) while later quarters compute. Host casts to f32.

Self-contained: hardcodes all shapes; requires only the concourse stack.
"""

import numpy as np
import ml_dtypes

try:
    import axon_prof

    axon_prof.install()
except Exception:
    pass

import concourse.mybir as mybir
import concourse.tile as tile
from concourse import bacc
from concourse import bass_utils

B, L, E = 4, 2048, 1024
H, DK = 16, 64
H8 = 8                      # heads per core
F = H8 * 3 * DK             # qkv features per core = 1536
FO = H8 * DK                # attn-out features per core = 512
NCORES = 8
Q4 = L // 4                 # 512 queries per quarter
Q8 = Q4 // 2                # 256 tokens scattered to each pair member

# ft-tile order: Q tiles 0..3 (head-pairs), K tiles 4..7, V tiles 8..11.
# Tiles {0,4,8} (head-pair 0) ship in w_pre so block (0,0) starts early.
PRE_FTS = (0, 4, 8)
REST_FTS = (1, 2, 3, 5, 6, 7, 9, 10, 11)
PRE_IDX = {ft: i for i, ft in enumerate(PRE_FTS)}
REST_IDX = {ft: i for i, ft in enumerate(REST_FTS)}

f32 = mybir.dt.float32
bf16 = mybir.dt.bfloat16
Exp = mybir.ActivationFunctionType.Exp
MUL = mybir.AluOpType.mult
ADD = mybir.AluOpType.add

_CACHE = {}


def build_nc():
    nc = bacc.Bacc("TRN2", target_bir_lowering=False, debug=False, num_devices=NCORES)

    # weight tensors arrive host-prearranged in SBUF layout (partition-major)
    # so every input DMA is contiguous per partition at full HBM rate
    x = nc.dram_tensor("x", [E, L], bf16, kind="ExternalInput")
    w_pre = nc.dram_tensor("w_pre", [128, 8 * 384], bf16, kind="ExternalInput")
    w_rest = nc.dram_tensor("w_rest", [128, 8 * 1152], bf16, kind="ExternalInput")
    b_qkv = nc.dram_tensor("b_qkv", [128, 12], f32, kind="ExternalInput")
    w_fc = nc.dram_tensor("w_fc", [128, 4 * E], bf16, kind="ExternalInput")
    b_fc = nc.dram_tensor("b_fc", [1, E], f32, kind="ExternalInput")
    # 4 quarters x 256 scattered tokens each; host casts bf16 -> f32
    out = nc.dram_tensor("out", [L // 2, E], bf16, kind="ExternalOutput")

    PAIRS = [[0, 1], [2, 3], [4, 5], [6, 7]]

    with tile.TileContext(nc) as tc:
        with (
            tc.tile_pool(name="persist", bufs=1) as pp,
            tc.tile_pool(name="work", bufs=2) as wp,
            tc.tile_pool(name="ys", bufs=3) as yp_pool,
            tc.tile_pool(name="stp", bufs=2, space="PSUM") as stp,
            tc.tile_pool(name="avp", bufs=1, space="PSUM") as avp,
            tc.tile_pool(name="qp", bufs=2, space="PSUM") as qp,
            tc.tile_pool(name="dram", bufs=1, space="DRAM") as dram,
        ):
            # ---- persistent SBUF ----
            xT = pp.tile([128, 8, L], bf16, tag="xT")          # X^T  4 MiB
            wqp = pp.tile([128, 8, 384], bf16, tag="wqp")
            wqr = pp.tile([128, 8, 1152], bf16, tag="wqr")
            bq = pp.tile([128, 12], f32, tag="bq")
            wfc = pp.tile([128, 4, E], bf16, tag="wfc")        # 1 MiB
            bias = pp.tile([128, E], f32, tag="bias")          # 0.5 MiB
            qt = pp.tile([128, 4, L], bf16, tag="qt")          # Q^T 2 MiB
            kt = pp.tile([128, 4, L], bf16, tag="kt")          # K^T 2 MiB
            vt = pp.tile([128, 4, L], bf16, tag="vt")          # V^T 2 MiB
            # V natural layout, 80-elem stride; col 64 holds the ones column
            # so AV matmuls with lhsT [V|1] (M=65) produce rowsums for free
            v = pp.tile([128, H8, 16, 80], bf16, tag="v")      # 2.5 MiB
            onT = pp.tile([128, 4, L], bf16, tag="onT")        # attn out^T 2 MiB

            rs_in = [
                dram.tile([Q4, E], bf16, name=f"rs_in{i}", tag=f"rs_in{i}")
                for i in range(4)
            ]
            rs_out = [
                dram.tile([Q8, E], bf16, name=f"rs_out{i}", tag=f"rs_out{i}")
                for i in range(4)
            ]

            # ---- input DMAs, all on the sync (SP) queue: the ACT queue must
            # stay empty or exp issue stalls behind DMA triggers. Small
            # tensors go first so nothing waits behind a bulk transfer;
            # wfc is deferred into the background schedule. ----
            nc.sync.dma_start(wqp[:], w_pre[:])
            nc.sync.dma_start(bq[:], b_qkv[:])
            bfc_row = pp.tile([1, E], f32, tag="bfc_row")
            nc.sync.dma_start(bfc_row[:], b_fc[:])
            for e in range(8):
                nc.sync.dma_start(xT[:, e, :], x[e * 128 : (e + 1) * 128, :])
            nc.sync.dma_start(wqr[:], w_rest[:])
            nc.gpsimd.partition_broadcast(bias[:], bfc_row[:])
            nc.vector.memset(v[:, :, :, 64:65], 1.0)

            def wq_sl(ft):
                if ft in PRE_IDX:
                    i = PRE_IDX[ft]
                    return lambda kc: wqp[:, kc, i * 128 : (i + 1) * 128]
                i = REST_IDX[ft]
                return lambda kc: wqr[:, kc, i * 128 : (i + 1) * 128]

            # ---- emitters: every matmul is a (64,128)-shape op; pairs at
            # positions (0,0)/(64,0) with identical moving columns fuse ----
            def qkv_chain(ft, tb):
                ps = qp.tile([128, 512], f32, tag="ps", name="ps")
                wa = wq_sl(ft)
                tsl = slice(tb * 512, (tb + 1) * 512)
                for kc in range(8):
                    nc.tensor.matmul(
                        ps[:], wa(kc), xT[:, kc, tsl],
                        start=(kc == 0), stop=(kc == 7),
                    )
                if ft < 4:
                    dst = qt[:, ft, tsl]
                elif ft < 8:
                    dst = kt[:, ft - 4, tsl]
                else:
                    dst = vt[:, ft - 8, tsl]
                nc.vector.tensor_scalar_add(dst, ps[:], bq[:, ft : ft + 1])

            def v_transpose(p, tb):
                # V^T -> V (token-major) via xbar transpose, per head, per
                # 512-token slice (4 key-chunks)
                for h in (2 * p, 2 * p + 1):
                    nc.sync.dma_start_transpose(
                        v[:, h, tb * 4 : (tb + 1) * 4, 0:DK],
                        vt[(h % 2) * 64 : (h % 2) * 64 + 64, p, tb * 512 : (tb + 1) * 512],
                    )

            def fc_chain(qq, tc_i, e2):
                # tokens (qq*4 + tc_i)*128 .. +128, output cols e2*512 .. +512
                t0 = (qq * 4 + tc_i) * 128
                esl = slice(e2 * 512, (e2 + 1) * 512)
                yp = qp.tile([128, 512], f32, tag="ps", name="yp")
                for c in range(4):
                    nc.tensor.matmul(
                        yp[:], onT[:, c, t0 : t0 + 128], wfc[:, c, esl],
                        start=(c == 0), stop=(c == 3),
                    )
                ys = yp_pool.tile([128, 512], bf16, tag="ys", name="ys")
                nc.vector.tensor_tensor(ys[:], yp[:], bias[:, esl], op=ADD)
                nc.sync.dma_start(
                    rs_in[qq][tc_i * 128 : (tc_i + 1) * 128, esl], ys[:]
                )

            def rs_quarter(qq):
                nc.gpsimd.collective_compute(
                    "ReduceScatter",
                    ADD,
                    replica_groups=PAIRS,
                    ins=[rs_in[qq].opt()],
                    outs=[rs_out[qq].opt()],
                )
                # deferred: the SWDGE copy for the PREVIOUS quarter (its RS is
                # long done, so this trigger never blocks the Pool stream)
                if qq > 0:
                    nc.gpsimd.dma_start(
                        out[(qq - 1) * Q8 : qq * Q8, :], rs_out[qq - 1][:]
                    )

            # ---- background schedule (due_step, fn); run at END of step s ----
            bg = []
            for tb in (1, 2, 3):
                bg.append((4 * tb - 2, lambda tb=tb: qkv_chain(4, tb)))
                bg.append((4 * tb - 1, lambda tb=tb: (qkv_chain(8, tb), v_transpose(0, tb))))
            for p in (1, 2, 3):
                bg.append((16 * p - 4, lambda p=p: qkv_chain(p, 0)))
                for tb in range(4):
                    bg.append((16 * p + 4 * tb - 2, lambda p=p, tb=tb: qkv_chain(4 + p, tb)))
                    bg.append(
                        (16 * p + 4 * tb - 1,
                         lambda p=p, tb=tb: (qkv_chain(8 + p, tb), v_transpose(p, tb)))
                    )
            bg.append((40, lambda: nc.sync.dma_start(wfc[:], w_fc[:])))
            for tb in (1, 2, 3):
                for p in range(4):
                    bg.append((64 * tb + 16 * p - 4, lambda p=p, tb=tb: qkv_chain(p, tb)))
            for qq in range(3):
                items = [(tc_i, e2) for tc_i in range(4) for e2 in range(2)]
                for i, (tc_i, e2) in enumerate(items):
                    due = 64 * qq + 66 + 4 * i
                    is_last = i == len(items) - 1
                    def fitem(qq=qq, tc_i=tc_i, e2=e2, is_last=is_last):
                        fc_chain(qq, tc_i, e2)
                        if is_last:
                            rs_quarter(qq)
                    bg.append((due, fitem))
            bg.sort(key=lambda t: t[0])
            bg_i = [0]

            def run_due_bg(s):
                while bg_i[0] < len(bg) and bg[bg_i[0]][0] <= s:
                    bg[bg_i[0]][1]()
                    bg_i[0] += 1

            # ---- prelude: head-pair 0, first 512 tokens ----
            qkv_chain(0, 0)
            qkv_chain(4, 0)
            qkv_chain(8, 0)
            v_transpose(0, 0)

            # ---- attention pipeline: 256 steps (1 key-tile x 2 heads) ----
            pts = {}
            av_tiles = {}

            def emit_av(s2):
                b_i, kk = divmod(s2, 16)
                qq, j = divmod(b_i, 4)
                if kk == 0:
                    av_tiles[b_i] = (
                        avp.tile([128, 512], f32, tag="avh0", name="avh0"),
                        avp.tile([128, 512], f32, tag="avh1", name="avh1"),
                    )
                pt = pts.pop(s2)
                for u, av in zip((0, 1), av_tiles[b_i]):
                    h = 2 * j + u
                    usl = slice(u * 512, (u + 1) * 512)
                    nc.tensor.matmul(
                        av[0:65, :],
                        v[:, h, kk, 0:65],
                        pt[:, usl],
                        start=(kk == 0),
                        stop=(kk == 15),
                    )
                if kk == 15:
                    evict_block(b_i, *av_tiles.pop(b_i))

            def evict_block(b_i, a0, a1):
                qq, j = divmod(b_i, 4)
                qsl = slice(qq * Q4, (qq + 1) * Q4)
                # one PSUM-read per av bank frees them for the next block
                comb = wp.tile([128, 1024], f32, tag="comb", name="comb")
                nc.vector.tensor_copy(comb[0:65, 0:512], a0[0:65, :])
                nc.vector.tensor_copy(comb[0:65, 512:1024], a1[0:65, :])
                # sums sit on partition 64; shift to partition 0, reciprocal,
                # broadcast; normalize on gpsimd (SBUF-only ops) to spare DVE
                srs = wp.tile([128, 2048], f32, tag="srs", name="srs")
                nc.sync.dma_start(srs[0:1, 0:1024], comb[64:65, 0:1024])
                nc.vector.reciprocal_approx_fast(
                    srs[0:1, 1024:2048], srs[0:1, 0:1024]
                )
                R = wp.tile([128, 1024], f32, tag="R", name="R")
                nc.gpsimd.partition_broadcast(R[:], srs[0:1, 1024:2048])
                nc.gpsimd.tensor_tensor(
                    onT[0:64, j, qsl], comb[0:64, 0:512], R[0:64, 0:512], op=MUL
                )
                tmp = wp.tile([64, 512], bf16, tag="tmp", name="tmp")
                nc.gpsimd.tensor_tensor(
                    tmp[:], comb[0:64, 512:1024], R[0:64, 512:1024], op=MUL
                )
                nc.sync.dma_start(onT[64:128, j, qsl], tmp[:])

            for s in range(256):
                b_i, kk = divmod(s, 16)
                qq, j = divmod(b_i, 4)
                qsl = slice(qq * Q4, (qq + 1) * Q4)
                ksl = slice(kk * 128, (kk + 1) * 128)
                st = stp.tile([128, 1024], f32, tag="st", name="st")
                nc.tensor.matmul(
                    st[:, 0:512], kt[0:64, j, ksl], qt[0:64, j, qsl],
                    start=True, stop=True,
                )
                nc.tensor.matmul(
                    st[:, 512:1024], kt[64:128, j, ksl], qt[64:128, j, qsl],
                    start=True, stop=True,
                )
                pt = wp.tile([128, 1024], bf16, tag="pt", bufs=3, name="pt")
                nc.scalar.activation(pt[:], st[:], Exp, scale=0.125)
                pts[s] = pt
                if s > 0:
                    emit_av(s - 1)
                run_due_bg(s)
            emit_av(255)

            # ---- tail: FC + RS for the last quarter ----
            run_due_bg(10**9)
            for tc_i in range(4):
                for e2 in range(2):
                    fc_chain(3, tc_i, e2)
            rs_quarter(3)
            nc.gpsimd.dma_start(out[2 * Q8 : 3 * Q8, :], rs_out[2][:])
            nc.gpsimd.dma_start(out[3 * Q8 : 4 * Q8, :], rs_out[3][:])

    nc.finalize()
    return nc


def _prep_inputs(X, W_qkv, b_qkv, W_fc, b_fc):
    """Host-side shard + permute + cast. Returns in_maps for 8 cores."""
    X = np.asarray(X, dtype=np.float32)
    W_qkv = np.asarray(W_qkv, dtype=np.float32)
    b_qkv = np.asarray(b_qkv, dtype=np.float32)
    W_fc = np.asarray(W_fc, dtype=np.float32)
    b_fc = np.asarray(b_fc, dtype=np.float32)

    pre_cols = np.concatenate([np.arange(ft * 128, (ft + 1) * 128) for ft in PRE_FTS])
    rest_cols = np.concatenate([np.arange(ft * 128, (ft + 1) * 128) for ft in REST_FTS])

    in_maps = []
    bfc_half = (0.5 * b_fc).astype(np.float32).reshape(1, E)
    for c in range(NCORES):
        b, g = divmod(c, 2)
        heads = np.arange(g * H8, (g + 1) * H8)
        # column order: all Q feats (head-major), then K, then V
        cols = np.concatenate(
            [
                np.concatenate([h * 3 * DK + off + np.arange(DK) for h in heads])
                for off in (0, DK, 2 * DK)
            ]
        )
        wq_sh = W_qkv[:, cols].astype(ml_dtypes.bfloat16)
        bq_sh = b_qkv[cols].astype(np.float32).reshape(12, 128).T.copy()
        wfc_sh = W_fc[g * FO : (g + 1) * FO, :].astype(ml_dtypes.bfloat16)

        def sbuf_layout(arr, width):
            # [(a p), f] -> [p, (a f)] so the device DMA is contiguous
            a = arr.shape[0] // 128
            return np.ascontiguousarray(
                arr.reshape(a, 128, width).transpose(1, 0, 2).reshape(128, a * width)
            )

        in_maps.append(
            {
                "x": np.ascontiguousarray(X[b].T).astype(ml_dtypes.bfloat16),
                "w_pre": sbuf_layout(wq_sh[:, pre_cols], 384),
                "w_rest": sbuf_layout(wq_sh[:, rest_cols], 1152),
                "b_qkv": np.ascontiguousarray(bq_sh),
                "w_fc": sbuf_layout(wfc_sh, E),
                "b_fc": bfc_half,
            }
        )
    return in_maps


def run_kernel(inputs, trace=False):
    if "nc" not in _CACHE:
        _CACHE["nc"] = build_nc()
    nc = _CACHE["nc"]
    in_maps = _prep_inputs(**inputs)
    res = bass_utils.run_bass_kernel_spmd(
        nc, in_maps, core_ids=list(range(NCORES)), trace=trace
    )
    Y = np.empty((B, L, E), dtype=np.float32)
    for c in range(NCORES):
        b, g = divmod(c, 2)
        o = np.asarray(res.results[c]["out"]).astype(np.float32)
        for qq in range(4):
            Y[b, qq * Q4 + g * Q8 : qq * Q4 + (g + 1) * Q8, :] = o[
                qq * Q8 : (qq + 1) * Q8
            ]
    return Y, res


def kernel(X, W_qkv, b_qkv, W_fc, b_fc):
    Y, _ = run_kernel(
        dict(X=X, W_qkv=W_qkv, b_qkv=b_qkv, W_fc=W_fc, b_fc=b_fc), trace=False
    )
    return Y
